# revision 11
# baseline (speedup 1.0000x reference)
"""Trainium2 Bass kernel for nn_AttentionBlock (GroupNorm + QKV + MHA).

Sharding: tensor-parallel over the H=8 heads, one head per NeuronCore.
Each core:
  - loads x^T (C on partitions) once,
  - computes GroupNorm statistics with bn_stats + tiny indicator matmuls,
  - folds the (data-dependent) GroupNorm affine INTO the QKV weights:
      xn = x*A + B  =>  qkv = x @ (A*W) + (b + B@W)
  - computes Q^T/K^T in two packed layouts [Q;K] and [K;Q] so that the
    k=64 QK^T matmuls can run as concurrent 64x128 row-tiles (tile_position),
  - softmax without max-subtraction (scores are O(1) here), denominator via
    a ones-column appended to V in the P^T-orientation AV matmul,
  - all matmuls in float32r (full PE rate, ~1.5e-4 per-matmul error).
Host side only reorders/permutes arrays and concatenates per-head outputs.
"""
import sys

try:
    import concourse.bass as bass  # noqa: F401
except ImportError:
    sys.path.insert(0, "/opt/trn_rl_repo")
    import concourse.bass as bass  # noqa: F401

import numpy as np
import concourse.tile as tile
from concourse import bacc, mybir
from concourse.bass_utils import run_bass_kernel_spmd

F32 = mybir.dt.float32
F32R = mybir.dt.float32r
AF = mybir.ActivationFunctionType
OP = mybir.AluOpType

T = 4096          # tokens (64*64)
C = 512           # channels
H = 8             # heads
CH = C // H       # 64 head dim
G = 32            # groupnorm groups
EPS = 1e-6
QKS = float(CH) ** -0.25   # scale applied to q and k each
NCORES = 8
TC = 512          # t-chunk width
NTC = T // TC     # 8 chunks
NSB = T // 128    # 32 s-blocks


def build_nc():
    nc = bacc.Bacc()
    xt = nc.declare_dram_parameter("xt", [C, T], F32R, isOutput=False)
    wqk = nc.declare_dram_parameter("wqk", [C, 128], F32R, isOutput=False)
    wkq = nc.declare_dram_parameter("wkq", [C, 128], F32R, isOutput=False)
    wv = nc.declare_dram_parameter("wv", [C, CH], F32R, isOutput=False)
    bqk = nc.declare_dram_parameter("bqk", [128, 1], F32, isOutput=False)
    bkq = nc.declare_dram_parameter("bkq", [128, 1], F32, isOutput=False)
    bv = nc.declare_dram_parameter("bv", [1, CH], F32, isOutput=False)
    gns = nc.declare_dram_parameter("gns", [128, 4], F32, isOutput=False)
    gnb = nc.declare_dram_parameter("gnb", [128, 4], F32, isOutput=False)
    ind = nc.declare_dram_parameter("ind", [128, 8], F32R, isOutput=False)
    indt = nc.declare_dram_parameter("indt", [8, 128], F32R, isOutput=False)
    outT = nc.declare_dram_parameter("outT", [CH, T], F32, isOutput=True)

    with tile.TileContext(nc) as tc:
        _build_body(nc, tc, xt, wqk, wkq, wv, bqk, bkq, bv, gns, gnb, ind,
                    indt, outT)
    nc.finalize()
    return nc


def _build_body(nc, tc, xt, wqk, wkq, wv, bqk, bkq, bv, gns, gnb, ind, indt,
                outT):
    from contextlib import ExitStack
    ctx = ExitStack()
    with ctx:
        const = ctx.enter_context(tc.tile_pool(name="const", bufs=1))
        big = ctx.enter_context(tc.tile_pool(name="big", bufs=1))
        work = ctx.enter_context(tc.tile_pool(name="work", bufs=2))
        ppool = ctx.enter_context(tc.tile_pool(name="ppool", bufs=3))
        ps_pair = ctx.enter_context(tc.tile_pool(name="ps_pair", bufs=2, space="PSUM"))
        ps_o = ctx.enter_context(tc.tile_pool(name="ps_o", bufs=2, space="PSUM"))
        ps_sm = ctx.enter_context(tc.tile_pool(name="ps_sm", bufs=2, space="PSUM"))

        # ---------------- loads ----------------
        xtk = []
        for k in range(4):
            t = big.tile([128, T], F32R, tag=f"xt{k}")
            nc.sync.dma_start(out=t, in_=xt[128 * k:128 * (k + 1), :])
            xtk.append(t)
        wqk_raw, wkq_raw, wv_raw = [], [], []
        for k in range(4):
            a = const.tile([128, 128], F32R, tag=f"wqkr{k}")
            nc.sync.dma_start(out=a, in_=wqk[128 * k:128 * (k + 1), :])
            wqk_raw.append(a)
            b = const.tile([128, 128], F32R, tag=f"wkqr{k}")
            nc.sync.dma_start(out=b, in_=wkq[128 * k:128 * (k + 1), :])
            wkq_raw.append(b)
            v = const.tile([128, CH], F32R, tag=f"wvr{k}")
            nc.sync.dma_start(out=v, in_=wv[128 * k:128 * (k + 1), :])
            wv_raw.append(v)
        bqk_t = const.tile([128, 1], F32, tag="bqk")
        nc.sync.dma_start(out=bqk_t, in_=bqk[:, :])
        bkq_t = const.tile([128, 1], F32, tag="bkq")
        nc.sync.dma_start(out=bkq_t, in_=bkq[:, :])
        bv_t = const.tile([1, CH], F32, tag="bv")
        nc.sync.dma_start(out=bv_t, in_=bv[:, :])
        gns_t = const.tile([128, 4], F32, tag="gns")
        nc.sync.dma_start(out=gns_t, in_=gns[:, :])
        gnb_t = const.tile([128, 4], F32, tag="gnb")
        nc.sync.dma_start(out=gnb_t, in_=gnb[:, :])
        ind_t = const.tile([128, 8], F32R, tag="ind")
        nc.sync.dma_start(out=ind_t, in_=ind[:, :])
        indt_t = const.tile([8, 128], F32R, tag="indt")
        nc.sync.dma_start(out=indt_t, in_=indt[:, :])

        # ones helpers (memset can't write f32r directly)
        ones_m_f = const.tile([1, 128], F32, tag="ones_m_f")
        nc.vector.memset(ones_m_f, 1.0)
        ones_m = const.tile([1, 128], F32R, tag="ones_m")
        nc.vector.tensor_copy(out=ones_m, in_=ones_m_f)
        ones_p_f = const.tile([128, 1], F32, tag="ones_p_f")
        nc.vector.memset(ones_p_f, 1.0)
        ones_p = const.tile([128, 1], F32R, tag="ones_p")
        nc.vector.tensor_copy(out=ones_p, in_=ones_p_f)

        # ---------------- phase A: groupnorm stats -> folded weights ------
        RS = work.tile([128, 8], F32R, tag="RS")
        tmp1 = work.tile([128, 1], F32, tag="tmp1")
        for k in range(4):
            st = work.tile([128, 8, 6], F32, tag="st")
            for sub in range(8):
                nc.vector.bn_stats(out=st[:, sub, :],
                                   in_=xtk[k][:, 512 * sub:512 * (sub + 1)])
            mv = work.tile([128, 2], F32, tag="mv")
            nc.vector.bn_aggr(out=mv, in_=st)
            # RS[:, 2k] = mean ; RS[:, 2k+1] = var + mean^2 = E[x^2]
            nc.vector.tensor_copy(out=RS[:, 2 * k:2 * k + 1], in_=mv[:, 0:1])
            nc.vector.tensor_mul(out=tmp1, in0=mv[:, 0:1], in1=mv[:, 0:1])
            nc.vector.tensor_add(out=RS[:, 2 * k + 1:2 * k + 2], in0=tmp1,
                                 in1=mv[:, 1:2])
        psG = ps_sm.tile([128, 8], F32, tag="sm")
        nc.tensor.matmul(out=psG[0:8, :], lhsT=ind_t, rhs=RS, start=True,
                         stop=True)
        # group stats (8 groups-per-tile x 4 tiles)
        gm = work.tile([8, 4], F32, tag="gm")
        nc.vector.tensor_scalar_mul(out=gm, in0=psG[0:8, 0:8:2], scalar1=1.0 / 16)
        ge2 = work.tile([8, 4], F32, tag="ge2")
        nc.vector.tensor_scalar_mul(out=ge2, in0=psG[0:8, 1:8:2], scalar1=1.0 / 16)
        gv = work.tile([8, 4], F32, tag="gv")
        t8 = work.tile([8, 4], F32, tag="t8")
        nc.vector.tensor_mul(out=t8, in0=gm, in1=gm)
        nc.vector.tensor_sub(out=gv, in0=ge2, in1=t8)
        # rstd = rsqrt(var+eps), one Newton step for accuracy
        gve = work.tile([8, 4], F32, tag="gve")
        nc.vector.tensor_scalar_add(out=gve, in0=gv, scalar1=EPS)
        sq0 = work.tile([8, 4], F32, tag="sq0")
        nc.scalar.activation(out=sq0, in_=gve, func=AF.Sqrt)
        r0 = work.tile([8, 4], F32, tag="r0")
        nc.vector.reciprocal(out=r0, in_=sq0)
        nc.vector.tensor_mul(out=t8, in0=r0, in1=r0)
        nc.vector.tensor_mul(out=t8, in0=t8, in1=gve)
        nc.vector.tensor_scalar(out=t8, in0=t8, scalar1=-0.5, scalar2=1.5,
                                op0=OP.mult, op1=OP.add)
        grstd = work.tile([8, 4], F32, tag="grstd")
        nc.vector.tensor_mul(out=grstd, in0=r0, in1=t8)
        # G[:, 2k] = gmean_k ; G[:, 2k+1] = grstd_k
        Gt = work.tile([8, 8], F32R, tag="Gt")
        nc.vector.tensor_copy(out=Gt[:, 0:8:2], in_=gm)
        nc.vector.tensor_copy(out=Gt[:, 1:8:2], in_=grstd)
        psB = ps_sm.tile([128, 8], F32, tag="sm")
        nc.tensor.matmul(out=psB, lhsT=indt_t, rhs=Gt, start=True, stop=True)
        # per-channel A = rstd*gn_scale ; B = gn_bias - mean*A
        Av = work.tile([128, 4], F32, tag="Av")
        nc.vector.tensor_mul(out=Av, in0=psB[:, 1:8:2], in1=gns_t)
        t128 = work.tile([128, 4], F32, tag="t128")
        nc.vector.tensor_mul(out=t128, in0=psB[:, 0:8:2], in1=Av)
        Bc = work.tile([128, 4], F32, tag="Bc")
        nc.vector.tensor_sub(out=Bc, in0=gnb_t, in1=t128)
        Br = work.tile([128, 4], F32R, tag="Br")
        nc.vector.tensor_copy(out=Br, in_=Bc)
        Bsr = work.tile([128, 4], F32R, tag="Bsr")
        nc.vector.tensor_scalar_mul(out=Bsr, in0=Bc, scalar1=QKS)
        # column-duplicated copy: fp32r matmuls need an even moving dim
        Bsr2 = work.tile([128, 8], F32R, tag="Bsr2")
        nc.vector.tensor_copy(out=Bsr2[:, 0:8:2], in_=Bsr)
        nc.vector.tensor_copy(out=Bsr2[:, 1:8:2], in_=Bsr)
        Aqk = work.tile([128, 4], F32, tag="Aqk")
        nc.vector.tensor_scalar_mul(out=Aqk, in0=Av, scalar1=QKS)
        # scaled weights
        wqk_s, wkq_s, wv_s = [], [], []
        for k in range(4):
            a = const.tile([128, 128], F32R, tag=f"wqks{k}")
            nc.vector.tensor_scalar_mul(out=a, in0=wqk_raw[k],
                                        scalar1=Aqk[:, k:k + 1])
            wqk_s.append(a)
            b = const.tile([128, 128], F32R, tag=f"wkqs{k}")
            nc.vector.tensor_scalar_mul(out=b, in0=wkq_raw[k],
                                        scalar1=Aqk[:, k:k + 1])
            wkq_s.append(b)
            v = const.tile([128, CH], F32R, tag=f"wvs{k}")
            nc.vector.tensor_scalar_mul(out=v, in0=wv_raw[k],
                                        scalar1=Av[:, k:k + 1])
            wv_s.append(v)
        # folded biases: b'' = s*b_host + (s*B) @ W_raw   (qk)  /  b + B @ Wv
        psQKb = ps_sm.tile([128, 2], F32, tag="sm")
        for k in range(4):
            nc.tensor.matmul(out=psQKb, lhsT=wqk_raw[k],
                             rhs=Bsr2[:, 2 * k:2 * k + 2],
                             start=(k == 0), stop=(k == 3))
        bqk_dev = const.tile([128, 1], F32, tag="bqk_dev")
        nc.vector.tensor_add(out=bqk_dev, in0=psQKb[:, 0:1], in1=bqk_t)
        psKQb = ps_sm.tile([128, 2], F32, tag="sm")
        for k in range(4):
            nc.tensor.matmul(out=psKQb, lhsT=wkq_raw[k],
                             rhs=Bsr2[:, 2 * k:2 * k + 2],
                             start=(k == 0), stop=(k == 3))
        bkq_dev = const.tile([128, 1], F32, tag="bkq_dev")
        nc.vector.tensor_add(out=bkq_dev, in0=psKQb[:, 0:1], in1=bkq_t)
        psBv = ps_sm.tile([1, CH], F32, tag="sm")
        for k in range(4):
            nc.tensor.matmul(out=psBv, lhsT=Br[:, k:k + 1], rhs=wv_raw[k],
                             start=(k == 0), stop=(k == 3))
        bv_row = const.tile([1, CH], F32R, tag="bv_row")
        nc.vector.tensor_add(out=bv_row, in0=psBv, in1=bv_t)

        # ---------------- phase B: projections ----------------
        qk1 = big.tile([128, T], F32R, tag="qk1")   # [Q; K]
        kq2 = big.tile([128, T], F32R, tag="kq2")   # [K; Q]
        for cc in range(NTC):
            sl = bass.ts(cc, TC)
            ps1 = ps_pair.tile([128, 1024], F32, tag="pp")
            for k in range(4):
                nc.tensor.matmul(out=ps1[:, 0:512], lhsT=wqk_s[k],
                                 rhs=xtk[k][:, sl], start=(k == 0),
                                 stop=(k == 3))
            for k in range(4):
                nc.tensor.matmul(out=ps1[:, 512:1024], lhsT=wkq_s[k],
                                 rhs=xtk[k][:, sl], start=(k == 0),
                                 stop=(k == 3))
            nc.vector.tensor_scalar(out=qk1[:, sl], in0=ps1[:, 0:512],
                                    scalar1=bqk_dev, scalar2=None, op0=OP.add)
            nc.vector.tensor_scalar(out=kq2[:, sl], in0=ps1[:, 512:1024],
                                    scalar1=bkq_dev, scalar2=None, op0=OP.add)
        # V (token-major) with ones column for the softmax denominator
        vt = big.tile([128, NSB * (CH + 1)], F32R, tag="vt")
        for ti in range(NSB):
            psV = ps_sm.tile([128, CH], F32, tag="sm")
            for k in range(4):
                nc.tensor.matmul(out=psV, lhsT=xtk[k][:, bass.ts(ti, 128)],
                                 rhs=wv_s[k], start=(k == 0), stop=False)
            nc.tensor.matmul(out=psV, lhsT=ones_m, rhs=bv_row, start=False,
                             stop=True)
            base = ti * (CH + 1)
            nc.vector.tensor_copy(out=vt[:, base:base + CH], in_=psV)
            nc.vector.tensor_copy(out=vt[:, base + CH:base + CH + 1],
                                  in_=ones_p)

        # ---------------- phase C: attention ----------------
        outsb = big.tile([CH, T], F32, tag="outsb")
        for cc in range(NTC):
            sl = bass.ts(cc, TC)
            psO = ps_o.tile([128, TC], F32, tag="psO")
            for pair in range(NSB // 2):
                sA, sB = 2 * pair, 2 * pair + 1
                pp = ps_pair.tile([128, 1024], F32, tag="pp")
                nc.tensor.matmul(out=pp[:, 0:512],
                                 lhsT=kq2[0:64, bass.ts(sA, 128)],
                                 rhs=qk1[0:64, sl], start=True, stop=True,
                                 tile_position=(0, 0))
                nc.tensor.matmul(out=pp[:, 512:1024],
                                 lhsT=qk1[64:128, bass.ts(sB, 128)],
                                 rhs=kq2[64:128, sl], start=True, stop=True,
                                 tile_position=(64, 0))
                P = ppool.tile([128, 1024], F32R, tag="P")
                nc.scalar.activation(out=P, in_=pp, func=AF.Exp)
                bA = sA * (CH + 1)
                bB = sB * (CH + 1)
                nc.tensor.matmul(out=psO[0:CH + 1, :],
                                 lhsT=vt[:, bA:bA + CH + 1], rhs=P[:, 0:512],
                                 start=(pair == 0), stop=False)
                nc.tensor.matmul(out=psO[0:CH + 1, :],
                                 lhsT=vt[:, bB:bB + CH + 1],
                                 rhs=P[:, 512:1024], start=False,
                                 stop=(pair == NSB // 2 - 1))
            rt = work.tile([1, TC], F32R, tag="rt")
            with nc.allow_low_precision(reason="f32r recip feeds PE broadcast"):
                nc.vector.reciprocal(out=rt, in_=psO[CH:CH + 1, :])
            psR = ps_sm.tile([CH, TC], F32, tag="sm")
            nc.tensor.matmul(out=psR, lhsT=ones_m[:, 0:CH], rhs=rt,
                             start=True, stop=True)
            rb = work.tile([CH, TC], F32, tag="rb")
            nc.scalar.copy(out=rb, in_=psR)
            nc.vector.tensor_mul(out=outsb[:, sl], in0=psO[0:CH, :], in1=rb)
        nc.sync.dma_start(out=outT[:, :], in_=outsb)


_NC_CACHE = None
TRACE = False          # set True (e.g. from test.py) to capture an NTFF profile
TMPDIR = None          # set to a path to keep NEFF/NTFF artifacts
LAST_RESULT = None     # BassKernelResults of the most recent kernel() call


def _get_nc():
    global _NC_CACHE
    if _NC_CACHE is None:
        _NC_CACHE = build_nc()
    return _NC_CACHE


def kernel(x, gn_scale, gn_bias, w_qkv, b_qkv):
    x = np.asarray(x, dtype=np.float32)
    gn_scale = np.asarray(gn_scale, dtype=np.float32)
    gn_bias = np.asarray(gn_bias, dtype=np.float32)
    w_qkv = np.asarray(w_qkv, dtype=np.float32)
    b_qkv = np.asarray(b_qkv, dtype=np.float32)

    spatial = x.shape[:-1]
    xt = np.ascontiguousarray(x.reshape(T, C).T)            # (C, T)
    w3 = w_qkv.reshape(C, C, 3)
    wq, wk, wv = w3[..., 0], w3[..., 1], w3[..., 2]          # (C, C) each
    b3 = b_qkv.reshape(C, 3)
    bq, bk, bvv = b3[:, 0], b3[:, 1], b3[:, 2]

    # indicator matrices: channel partition p -> group j = p // 16
    p_idx = np.arange(128)
    ind = (p_idx[:, None] // 16 == np.arange(8)[None, :]).astype(np.float32)
    indt = np.ascontiguousarray(ind.T)
    gns_dev = np.ascontiguousarray(gn_scale.reshape(4, 128).T)
    gnb_dev = np.ascontiguousarray(gn_bias.reshape(4, 128).T)

    in_maps = []
    for h in range(H):
        hsl = slice(CH * h, CH * (h + 1))
        wq_h, wk_h, wv_h = wq[:, hsl], wk[:, hsl], wv[:, hsl]
        in_maps.append({
            "xt": xt,
            "wqk": np.ascontiguousarray(np.concatenate([wq_h, wk_h], axis=1)),
            "wkq": np.ascontiguousarray(np.concatenate([wk_h, wq_h], axis=1)),
            "wv": np.ascontiguousarray(wv_h),
            "bqk": (QKS * np.concatenate([bq[hsl], bk[hsl]])).reshape(128, 1),
            "bkq": (QKS * np.concatenate([bk[hsl], bq[hsl]])).reshape(128, 1),
            "bv": bvv[hsl].reshape(1, CH).copy(),
            "gns": gns_dev,
            "gnb": gnb_dev,
            "ind": ind,
            "indt": indt,
        })

    nc = _get_nc()
    res = run_bass_kernel_spmd(nc, in_maps, list(range(NCORES)), trace=TRACE,
                               tmpdir=TMPDIR)
    global LAST_RESULT
    LAST_RESULT = res
    out = np.empty((T, C), dtype=np.float32)
    for h in range(H):
        out[:, CH * h:CH * (h + 1)] = res.results[h]["outT"].T
    return out.reshape(spatial + (C,))
